# revision 1
# baseline (speedup 1.0000x reference)
"""Trainium2 Bass kernel for nn_CascadeGNN (cascade AGNN over 256 graphs).

Strategy (graph-sharded SPMD over 8 NeuronCores, 32 graphs/core):
  * All AGNN message passing is done densely per graph: edges within a graph
    are encoded as a dense [512,512] multiplicity (count) matrix Ct built on
    host from the int32 edge lists (pure topology/format conversion; all
    float compute runs on device).
  * AGNN without max-subtraction (softmax is shift-invariant; beta*cos is
    bounded), so per graph:
        cos  = hn^T hn + c * ir ir^T      (rank-1 term absorbs the per-graph
                                           broadcast query vector; c=|u_g|^2)
        W    = Ct * exp(beta*cos)         ([src, dst] layout)
        numT = h_nm^T W, den = 1^T W      (PSUM-accumulated matmuls)
        out  = num * (den>0)/max(den,eps) (column scaling via rank-1 matmul)
  * 32 query graphs (16 nodes each) are packed into one block-diagonal
    512-node graph and use the same code path (c=0).
  * The MLP second half (broadcast query features) collapses to the rank-1
    term mask (x) (B1^T u_g), never materialized per node.
All heavy matmuls run in bf16 with fp32 PSUM accumulation.
"""

import threading
from contextlib import ExitStack

import numpy as np
import ml_dtypes

import concourse.bass as bass
import concourse.mybir as mybir
import concourse.tile as tile
from concourse import bacc
from concourse.bass import ds, ts
from concourse.bass_utils import run_bass_kernel_spmd
from concourse.masks import make_identity

BF16 = mybir.dt.bfloat16
F32 = mybir.dt.float32
AF = mybir.ActivationFunctionType
ALU = mybir.AluOpType

# problem constants
B = 256
NPG = 512
NQPG = 16
IN, H, L, OUT = 64, 128, 2, 1
NCORES = 8
GPC = B // NCORES          # graphs per core (32)
N = NPG                    # dense block size for data graphs
NCH = N // 128             # 4 chunks of 128 src nodes


NQ_FIX = 512               # padded query block (32 graphs x 16 nodes)
G32 = NQ_FIX // NQPG       # 32 query slots


def build_program(gpc=GPC):
    """Build the per-core Bass/Tile program (identical on all 8 cores)."""
    n_nodes = gpc * NPG
    nq_blk = NQ_FIX

    nc = bacc.Bacc("TRN2", target_bir_lowering=False, debug=False,
                   num_devices=NCORES)

    io = {}
    io["xt"] = nc.dram_tensor("xt", [IN, n_nodes], BF16,
                              kind="ExternalInput").ap()
    io["xqt"] = nc.dram_tensor("xqt", [IN, NQ_FIX], BF16,
                               kind="ExternalInput").ap()
    io["ct"] = nc.dram_tensor("ct", [gpc + 1, NCH, 128, N], BF16,
                              kind="ExternalInput").ap()
    for nm, shp, dt in [
        ("wg", [IN, H], BF16), ("wq", [IN, H], BF16),
        ("bgc", [H, 1], F32), ("bqc", [H, 1], F32),
        ("betg", [L, H, 1], F32), ("betq", [L, H, 1], F32),
        ("a1", [L, H, H], BF16), ("b1t", [L, H, H], BF16),
        ("w2", [L, H, H], BF16),
        ("b1c", [L, H, 1], F32), ("b2c", [L, H, 1], F32),
        ("wp1", [H, H], BF16), ("wp2", [H, 1], BF16),
        ("bp1c", [H, 1], F32), ("bp2c", [1, 1], F32),
    ]:
        io[nm] = nc.dram_tensor(nm, shp, dt, kind="ExternalInput").ap()
    io["y"] = nc.dram_tensor("y", [1, gpc], F32, kind="ExternalOutput").ap()

    with tile.TileContext(nc) as tc:
        _emit(tc, nc, gpc, io)
    nc.compile()
    return nc


def _emit(tc, nc, gpc, io):
    n_nodes = gpc * NPG
    nq_blk = NQ_FIX
    nqch = nq_blk // 128

    ctx = ExitStack()
    with ctx:
        pconst = ctx.enter_context(tc.tile_pool(name="pconst", bufs=1))
        pstate = ctx.enter_context(tc.tile_pool(name="pstate", bufs=1))
        pct = ctx.enter_context(tc.tile_pool(name="pct", bufs=2))
        pwork = ctx.enter_context(tc.tile_pool(name="pwork", bufs=3))
        prow = ctx.enter_context(tc.tile_pool(name="prow", bufs=4))
        ps_cos = ctx.enter_context(
            tc.tile_pool(name="ps_cos", bufs=2, space="PSUM"))
        ps_big = ctx.enter_context(
            tc.tile_pool(name="ps_big", bufs=2, space="PSUM"))
        ps_acc = ctx.enter_context(
            tc.tile_pool(name="ps_acc", bufs=2, space="PSUM"))
        ps_row = ctx.enter_context(
            tc.tile_pool(name="ps_row", bufs=1, space="PSUM"))

        def const(name, shape, dtype):
            return pconst.tile(shape, dtype, name=name, tag=name)

        # ---- constants ----
        ident_f = const("ident_f", [128, 128], F32)
        make_identity(nc, ident_f[:])
        ones_col_bf = const("ones_col_bf", [128, 1], BF16)
        nc.vector.memset(ones_col_bf[:], 1.0)
        ones_row_bf = const("ones_row_bf", [1, 128], BF16)
        nc.vector.memset(ones_row_bf[:], 1.0)
        qeps = const("qeps", [1, 1], F32)
        nc.vector.memset(qeps[:], 1e-24)
        ones_512f = const("ones_512f", [1, N], F32)
        nc.vector.memset(ones_512f[:], 1.0)

        # ---- load weights into SBUF ----
        def load(name, ap_dram, shape, dtype):
            t = const(name, shape, dtype)
            nc.sync.dma_start(t[:], ap_dram)
            return t

        wg_s = load("wg_s", io["wg"][:], [IN, H], BF16)
        wq_s = load("wq_s", io["wq"][:], [IN, H], BF16)
        bgc_s = load("bgc_s", io["bgc"][:], [H, 1], F32)
        bqc_s = load("bqc_s", io["bqc"][:], [H, 1], F32)
        betg_s = [load(f"betg{l}", io["betg"][l], [H, 1], F32)
                  for l in range(L)]
        betq_s = [load(f"betq{l}", io["betq"][l], [H, 1], F32)
                  for l in range(L)]
        a1_s = [load(f"a1_{l}", io["a1"][l], [H, H], BF16) for l in range(L)]
        b1t_s = [load(f"b1t_{l}", io["b1t"][l], [H, H], BF16)
                 for l in range(L)]
        w2_s = [load(f"w2_{l}", io["w2"][l], [H, H], BF16) for l in range(L)]
        b1c_s = [load(f"b1c_{l}", io["b1c"][l], [H, 1], F32)
                 for l in range(L)]
        b2c_s = [load(f"b2c_{l}", io["b2c"][l], [H, 1], F32)
                 for l in range(L)]
        wp1_s = load("wp1_s", io["wp1"][:], [H, H], BF16)
        wp2_s = load("wp2_s", io["wp2"][:], [H, 1], BF16)
        bp1c_s = load("bp1c_s", io["bp1c"][:], [H, 1], F32)
        bp2c_s = load("bp2c_s", io["bp2c"][:], [1, 1], F32)

        xqt_s = load("xqt_s", io["xqt"][:], [IN, nq_blk], BF16)

        # query-block count matrix: resident for the whole kernel
        ctq_tile = const("ctq_tile", [128, NCH, N], BF16)
        nc.sync.dma_start(ctq_tile[:],
                          io["ct"][gpc].rearrange("c p f -> p c f"))

        # ---- persistent state ----
        def state(name, shape, dtype):
            return pstate.tile(shape, dtype, name=name, tag=name)

        HGT = state("HGT", [128, n_nodes], F32)           # h_g feature-major
        HGN = state("HGN", [128, gpc * NCH * 128], BF16)  # h_g node-major bf16
        HQT = state("HQT", [128, nq_blk], F32)
        HQN = state("HQN", [128, nqch * 128], BF16)
        HGS = state("HGS", [128, gpc], F32)
        u_bf = state("u_bf", [128, G32], BF16)
        c_row = state("c_row", [1, G32], F32)
        ce_row = state("ce_row", [1, G32], F32)
        vrow = state("vrow", [1, G32 * 128], BF16)

        def hgn_sl(g, sc):
            return HGN[:, ds((g * NCH + sc) * 128, 128)]

        def make_node_major(srcT, dst_ap, k):
            """srcT [128, k*128] f32 SBUF -> transpose -> dst bf16
            [128, k*128] (node-major)."""
            tpb = ps_big.tile([128, k * 128], F32, name="tpb", tag="big")
            for i in range(k):
                nc.tensor.transpose(tpb[:, ts(i, 128)], srcT[:, ts(i, 128)],
                                    ident_f[:])
            nc.vector.tensor_copy(dst_ap, tpb[:])

        # ---- initial projections ----
        for g in range(gpc):
            xg_t = pwork.tile([IN, N], BF16, name="xg_t", tag="xg")
            nc.sync.dma_start(xg_t[:], io["xt"][:, ts(g, N)])
            xg_ps = ps_big.tile([128, N], F32, name="xg_ps", tag="big")
            nc.tensor.matmul(xg_ps[:], wg_s[:], xg_t[:],
                             start=True, stop=True)
            nc.scalar.activation(HGT[:, ts(g, N)], xg_ps[:], AF.Identity,
                                 bias=bgc_s[:])
            make_node_major(HGT[:, ts(g, N)],
                            HGN[:, ds(g * NCH * 128, NCH * 128)], NCH)

        xq_ps = ps_big.tile([128, nq_blk], F32, name="xq_ps", tag="big")
        nc.tensor.matmul(xq_ps[:], wq_s[:], xqt_s[:], start=True, stop=True)
        nc.scalar.activation(HQT[:], xq_ps[:], AF.Identity, bias=bqc_s[:])
        make_node_major(HQT[:], HQN[:], nqch)

        def agnn(hT, h_nm_slices, ct_all_ap, beta_col, c_ap, nch, nn):
            """Dense AGNN block; returns (num_ps, dm, mask_bf)."""
            sq = pwork.tile([128, nn], BF16, name="sq", tag="sq")
            nc.gpsimd.tensor_mul(sq[:], hT, hT)
            nsq_ps = ps_row.tile([1, nn], F32, name="nsq_ps", tag="row")
            nc.tensor.matmul(nsq_ps[:], ones_col_bf[:], sq[:],
                             start=True, stop=True)
            ln_row = prow.tile([1, nn], F32, name="ln_row", tag="frow")
            bias = qeps[:] if c_ap is None else c_ap
            nc.scalar.activation(ln_row[:], nsq_ps[:], AF.Ln, bias=bias)
            ir_row = prow.tile([1, nn], BF16, name="ir_row", tag="brow")
            nc.scalar.activation(ir_row[:], ln_row[:], AF.Exp, scale=-0.5)
            cir_row = None
            if c_ap is not None:
                cir_row = prow.tile([1, nn], BF16, name="cir_row", tag="brow")
                nc.vector.tensor_scalar(cir_row[:], ir_row[:], c_ap, None,
                                        op0=ALU.mult)
            irb = pwork.tile([128, nn], BF16, name="irb", tag="irb")
            nc.gpsimd.partition_broadcast(irb[:], ir_row[:])
            hn = pwork.tile([128, nn], BF16, name="hn", tag="hn")
            nc.vector.tensor_tensor(hn[:], hT, irb[:], op=ALU.mult)

            num_ps = ps_acc.tile([128, nn], F32, name="num_ps", tag="acc")
            den_ps = ps_row.tile([1, nn], F32, name="den_ps", tag="row")
            for sc in range(nch):
                cos_ps = ps_cos.tile([128, nn], F32, name="cos_ps",
                                     tag="cos")
                nc.tensor.matmul(cos_ps[:], hn[:, ts(sc, 128)],
                                 hn[:], start=True,
                                 stop=(cir_row is None))
                if cir_row is not None:
                    nc.tensor.matmul(cos_ps[:],
                                     cir_row[:, ts(sc, 128)], ir_row[:],
                                     start=False, stop=True)
                ee = pwork.tile([128, nn], BF16, name="ee", tag="ee")
                nc.scalar.activation(ee[:], cos_ps[:], AF.Exp, scale=beta_col)
                wt = pwork.tile([128, nn], BF16, name="wt", tag="wt")
                nc.vector.tensor_tensor(wt[:], ee[:], ct_all_ap[:, sc, :],
                                        op=ALU.mult)
                nc.tensor.matmul(num_ps[:], h_nm_slices[sc], wt[:],
                                 start=(sc == 0), stop=(sc == nch - 1))
                nc.tensor.matmul(den_ps[:], ones_col_bf[:], wt[:],
                                 start=(sc == 0), stop=(sc == nch - 1))

            mbar = prow.tile([1, nn], F32, name="mbar", tag="frow")
            nc.vector.tensor_scalar(mbar[:], den_ps[:], 0.0, None,
                                    op0=ALU.is_le)
            t_row = prow.tile([1, nn], F32, name="t_row", tag="frow")
            nc.vector.tensor_tensor(t_row[:], mbar[:], den_ps[:], op=ALU.add)
            dmask_f = prow.tile([1, nn], F32, name="dmask_f", tag="frow")
            nc.vector.reciprocal(dmask_f[:], t_row[:])
            mask_bf = prow.tile([1, nn], BF16, name="mask_bf", tag="brow")
            nc.gpsimd.tensor_scalar(mask_bf[:], mbar[:], 0.0, None,
                                    op0=ALU.is_equal)
            dm = pwork.tile([128, nn], F32, name="dm", tag="dm")
            nc.gpsimd.partition_broadcast(dm[:], dmask_f[:])
            return num_ps, dm, mask_bf

        for l in range(L):
            # ---- query AGNN on the packed block-diagonal graph ----
            num_ps, dm, _ = agnn(
                HQT[:], [HQN[:, ts(sc, 128)] for sc in range(nqch)],
                ctq_tile[:], betq_s[l][:], None, nqch, nq_blk)
            nc.vector.tensor_tensor(HQT[:], num_ps[:], dm[:], op=ALU.mult)
            make_node_major(HQT[:], HQN[:], nqch)

            # ---- per-graph query aggregates: u, c = |u|^2, v = B1^T u ----
            u_f = pwork.tile([128, G32], F32, name="u_f", tag="uf")
            nc.vector.tensor_reduce(
                u_f[:], HQT[:].rearrange("p (g k) -> p g k", k=NQPG),
                axis=mybir.AxisListType.X, op=ALU.add)
            nc.vector.tensor_copy(u_bf[:], u_f[:])
            squ = pwork.tile([128, G32], BF16, name="squ", tag="uf")
            nc.gpsimd.tensor_mul(squ[:], u_f[:], u_f[:])
            c_ps = ps_row.tile([1, G32], F32, name="c_ps", tag="row")
            nc.tensor.matmul(c_ps[:], ones_col_bf[:], squ[:],
                             start=True, stop=True)
            nc.vector.tensor_copy(c_row[:], c_ps[:])
            nc.vector.tensor_scalar(ce_row[:], c_ps[:], 1e-24, None,
                                    op0=ALU.add)
            # v = B1^T u for all graphs at once; flatten [G32,128] rows to
            # partition-0 [1, G32*128] via one SBUF->SBUF DMA
            v_ps = ps_big.tile([128, G32], F32, name="v_ps", tag="big")
            nc.tensor.matmul(v_ps[:], b1t_s[l][:], u_bf[:],
                             start=True, stop=True)
            v_sb = pwork.tile([128, G32], F32, name="v_sb", tag="uf")
            nc.vector.tensor_copy(v_sb[:], v_ps[:])
            vt_ps = ps_big.tile([G32, 128], F32, name="vt_ps", tag="big")
            nc.tensor.transpose(vt_ps[:], v_sb[:], ident_f[:])
            vt32 = pwork.tile([G32, 128], BF16, name="vt32", tag="vt32")
            nc.vector.tensor_copy(vt32[:], vt_ps[:])
            nc.sync.dma_start(vrow[:], vt32[:])

            # ---- data graphs, processed in pairs: elementwise/row ops run
            # 1024-wide across both graphs; matmuls/MLP remain per-graph ----
            for p in range(gpc // 2):
                gA = 2 * p
                ctg2 = pct.tile([128, 2, NCH, N], BF16, name="ctg2", tag="ct")
                nc.sync.dma_start(
                    ctg2[:],
                    io["ct"][ds(gA, 2)].rearrange("g c p f -> p g c f"))

                hTp = HGT[:, ds(gA * N, 2 * N)]
                sqp = pwork.tile([128, 2 * N], BF16, name="sqp", tag="sq")
                nc.gpsimd.tensor_mul(sqp[:], hTp, hTp)
                nsq_ps = ps_row.tile([1, 2 * N], F32, name="nsq_ps",
                                     tag="row")
                for gi in range(2):
                    nc.tensor.matmul(nsq_ps[0:1, ds(gi * N, N)],
                                     ones_col_bf[:],
                                     sqp[:, ds(gi * N, N)],
                                     start=True, stop=False)
                    # per-graph c via a K=1,M=1 rank-0 matmul
                    nc.tensor.matmul(nsq_ps[0:1, ds(gi * N, N)],
                                     c_row[0:1, ds(gA + gi, 1)],
                                     ones_512f[:],
                                     start=False, stop=True)
                lnp = prow.tile([1, 2 * N], F32, name="lnp", tag="frow")
                nc.scalar.activation(lnp[:], nsq_ps[:], AF.Ln, bias=qeps[:])
                irp = prow.tile([1, 2 * N], BF16, name="irp", tag="brow")
                nc.scalar.activation(irp[:], lnp[:], AF.Exp, scale=-0.5)
                cirp = prow.tile([1, 2 * N], BF16, name="cirp", tag="brow")
                for gi in range(2):
                    nc.vector.tensor_scalar(cirp[0:1, ds(gi * N, N)],
                                            irp[0:1, ds(gi * N, N)],
                                            c_row[0:1, ds(gA + gi, 1)], None,
                                            op0=ALU.mult)
                irbp = pwork.tile([128, 2 * N], BF16, name="irbp", tag="irb")
                nc.gpsimd.partition_broadcast(irbp[:], irp[:])
                hnp = pwork.tile([128, 2 * N], BF16, name="hnp", tag="hn")
                nc.vector.tensor_tensor(hnp[:], hTp, irbp[:], op=ALU.mult)

                den_ps = ps_row.tile([1, 2 * N], F32, name="den_ps",
                                     tag="row")
                num_pss = []
                for gi in range(2):
                    num_ps = ps_acc.tile([128, N], F32, name="num_ps",
                                         tag="acc")
                    num_pss.append(num_ps)
                    for sc in range(NCH):
                        cos_ps = ps_cos.tile([128, N], F32, name="cos_ps",
                                             tag="cos")
                        nc.tensor.matmul(
                            cos_ps[:],
                            hnp[:, ds(gi * N + sc * 128, 128)],
                            hnp[:, ds(gi * N, N)],
                            start=True, stop=False)
                        nc.tensor.matmul(
                            cos_ps[:],
                            cirp[0:1, ds(gi * N + sc * 128, 128)],
                            irp[0:1, ds(gi * N, N)],
                            start=False, stop=True)
                        ee = pwork.tile([128, N], BF16, name="ee", tag="ee")
                        nc.scalar.activation(ee[:], cos_ps[:], AF.Exp,
                                             scale=betg_s[l][:])
                        wt = pwork.tile([128, N], BF16, name="wt", tag="wt")
                        nc.vector.tensor_tensor(
                            wt[:], ee[:], ctg2[:, gi, sc, :], op=ALU.mult)
                        nc.tensor.matmul(num_ps[:], hgn_sl(gA + gi, sc),
                                         wt[:], start=(sc == 0),
                                         stop=(sc == NCH - 1))
                        nc.tensor.matmul(den_ps[0:1, ds(gi * N, N)],
                                         ones_col_bf[:], wt[:],
                                         start=(sc == 0),
                                         stop=(sc == NCH - 1))

                mbar = prow.tile([1, 2 * N], F32, name="mbar", tag="frow")
                nc.vector.tensor_scalar(mbar[:], den_ps[:], 0.0, None,
                                        op0=ALU.is_le)
                t_row = prow.tile([1, 2 * N], F32, name="t_row", tag="frow")
                nc.vector.tensor_tensor(t_row[:], mbar[:], den_ps[:],
                                        op=ALU.add)
                dmask_f = prow.tile([1, 2 * N], F32, name="dmask_f",
                                    tag="frow")
                nc.vector.reciprocal(dmask_f[:], t_row[:])
                mask_bf = prow.tile([1, 2 * N], BF16, name="mask_bf",
                                    tag="brow")
                nc.gpsimd.tensor_scalar(mask_bf[:], mbar[:], 0.0, None,
                                        op0=ALU.is_equal)
                dmp = pwork.tile([128, 2 * N], F32, name="dmp", tag="dm")
                nc.gpsimd.partition_broadcast(dmp[:], dmask_f[:])

                for gi in range(2):
                    g = gA + gi
                    s1 = pwork.tile([128, N], BF16, name="s1", tag="s1")
                    nc.vector.tensor_tensor(s1[:], num_pss[gi][:],
                                            dmp[:, ds(gi * N, N)],
                                            op=ALU.mult)
                    z_ps = ps_big.tile([128, N], F32, name="z_ps", tag="big")
                    nc.tensor.matmul(z_ps[:], a1_s[l][:], s1[:],
                                     start=True, stop=False)
                    nc.tensor.matmul(z_ps[:], vrow[0:1, ts(g, 128)],
                                     mask_bf[0:1, ds(gi * N, N)],
                                     start=False, stop=True)
                    rz = pwork.tile([128, N], BF16, name="rz", tag="s1")
                    nc.scalar.activation(rz[:], z_ps[:], AF.Relu,
                                         bias=b1c_s[l][:])
                    h2_ps = ps_acc.tile([128, N], F32, name="h2_ps",
                                        tag="acc")
                    nc.tensor.matmul(h2_ps[:], w2_s[l][:], rz[:],
                                     start=True, stop=True)
                    nc.scalar.activation(HGT[:, ts(g, N)], h2_ps[:],
                                         AF.Identity, bias=b2c_s[l][:])
                    make_node_major(HGT[:, ts(g, N)],
                                    HGN[:, ds(g * NCH * 128, NCH * 128)],
                                    NCH)
                    if l == L - 1:
                        nc.vector.tensor_reduce(
                            HGS[:, ds(g, 1)], HGT[:, ts(g, N)],
                            axis=mybir.AxisListType.X, op=ALU.add)

        # ---- final predictor ----
        hgs_bf = pwork.tile([128, gpc], BF16, name="hgs_bf", tag="uf")
        nc.vector.tensor_copy(hgs_bf[:], HGS[:])
        z1_ps = ps_big.tile([128, gpc], F32, name="z1_ps", tag="big")
        nc.tensor.matmul(z1_ps[:], wp1_s[:], hgs_bf[:], start=True, stop=True)
        r1 = pwork.tile([128, gpc], BF16, name="r1", tag="uf")
        nc.scalar.activation(r1[:], z1_ps[:], AF.Relu, bias=bp1c_s[:])
        y_ps = ps_row.tile([1, gpc], F32, name="y_ps", tag="row")
        nc.tensor.matmul(y_ps[:], wp2_s[:], r1[:], start=True, stop=True)
        y_sb = prow.tile([1, gpc], F32, name="y_sb", tag="frow")
        nc.scalar.activation(y_sb[:], y_ps[:], AF.Identity, bias=bp2c_s[:])
        nc.sync.dma_start(io["y"][:], y_sb[:])


def _build_ct_np(src, dst, npb, nblocks):
    blk = src // npb
    s = src - blk * npb
    d = dst - blk * npb
    flat = blk * (npb * npb) + s * npb + d
    cnt = np.bincount(flat, minlength=nblocks * npb * npb)
    return cnt.reshape(nblocks, npb, npb)


_PROG_CACHE = {}
_PROG_LOCK = threading.Lock()


def _get_program(gpc=GPC):
    with _PROG_LOCK:
        if gpc not in _PROG_CACHE:
            _PROG_CACHE[gpc] = build_program(gpc)
        return _PROG_CACHE[gpc]


def _make_in_maps(inputs, gpc=GPC, ncores=NCORES):
    bf = ml_dtypes.bfloat16
    X = np.asarray(inputs["X"], np.float32)
    X_q = np.asarray(inputs["X_q"], np.float32)
    g_src = np.asarray(inputs["g_src"], np.int64)
    g_dst = np.asarray(inputs["g_dst"], np.int64)
    q_src = np.asarray(inputs["q_src"], np.int64)
    q_dst = np.asarray(inputs["q_dst"], np.int64)

    W1r = np.asarray(inputs["W1r"], np.float32)
    shared = {
        "wg": np.asarray(inputs["Wg"], np.float32).astype(bf),
        "wq": np.asarray(inputs["Wq"], np.float32).astype(bf),
        "bgc": np.asarray(inputs["bg"], np.float32).reshape(H, 1).copy(),
        "bqc": np.asarray(inputs["bq"], np.float32).reshape(H, 1).copy(),
        "betg": np.tile(
            np.asarray(inputs["betas_g"], np.float32).reshape(L, 1, 1),
            (1, H, 1)),
        "betq": np.tile(
            np.asarray(inputs["betas_q"], np.float32).reshape(L, 1, 1),
            (1, H, 1)),
        "a1": np.ascontiguousarray(W1r[:, :H, :]).astype(bf),
        "b1t": np.ascontiguousarray(W1r[:, H:, :]).astype(bf),
        "w2": np.asarray(inputs["W2r"], np.float32).astype(bf),
        "b1c": np.asarray(inputs["b1r"], np.float32).reshape(L, H, 1).copy(),
        "b2c": np.asarray(inputs["b2r"], np.float32).reshape(L, H, 1).copy(),
        "wp1": np.asarray(inputs["Wp1"], np.float32).astype(bf),
        "wp2": np.asarray(inputs["Wp2"], np.float32).astype(bf),
        "bp1c": np.asarray(inputs["bp1"], np.float32).reshape(H, 1).copy(),
        "bp2c": np.asarray(inputs["bp2"], np.float32).reshape(1, 1).copy(),
    }

    n = gpc * NPG
    nq = gpc * NQPG
    ne = n * 8
    nqe = nq * 8
    in_maps = []
    for c in range(ncores):
        xc = X[c * n:(c + 1) * n]
        xqc = X_q[c * nq:(c + 1) * nq]
        gs = g_src[c * ne:(c + 1) * ne] - c * n
        gd = g_dst[c * ne:(c + 1) * ne] - c * n
        qs = q_src[c * nqe:(c + 1) * nqe] - c * nq
        qd = q_dst[c * nqe:(c + 1) * nqe] - c * nq

        ct_g = _build_ct_np(gs, gd, NPG, gpc)       # [gpc, 512, 512]
        ct_q = _build_ct_np(qs, qd, NQPG, gpc)      # [gpc, 16, 16]
        ctq_blk = np.zeros((512, 512), np.int64)
        for g in range(gpc):
            ctq_blk[g * NQPG:(g + 1) * NQPG,
                    g * NQPG:(g + 1) * NQPG] = ct_q[g]

        ct_all = np.concatenate([ct_g, ctq_blk[None]], 0)
        ct_all = ct_all.reshape(gpc + 1, NCH, 128, N).astype(bf)

        m = dict(shared)
        m["xt"] = np.ascontiguousarray(xc.T).astype(bf)
        xqt = np.zeros((IN, 512), np.float32)
        xqt[:, :nq] = xqc.T
        m["xqt"] = xqt.astype(bf)
        m["ct"] = ct_all
        in_maps.append(m)
    return in_maps


def run(inputs, trace=False, gpc=GPC):
    nc = _get_program(gpc)
    in_maps = _make_in_maps(inputs, gpc=gpc)
    res = run_bass_kernel_spmd(nc, in_maps, list(range(NCORES)), trace=trace)
    ys = [res.results[c]["y"].reshape(-1) for c in range(NCORES)]
    out = np.concatenate(ys).astype(np.float32).reshape(B, OUT)
    return out, res


def kernel(**inputs) -> np.ndarray:
    out, _ = run(inputs, trace=False)
    return out



# revision 67
# speedup vs baseline: 1.8858x; 1.8858x over previous
"""Trainium2 Bass kernel for nn_CascadeGNN (cascade AGNN over 256 graphs).

Strategy (graph-sharded SPMD over 8 NeuronCores, 32 graphs/core):
  * All AGNN message passing is done densely per graph: edges within a graph
    are encoded as a dense [512,512] multiplicity (count) matrix Ct built on
    host from the int32 edge lists (pure topology/format conversion; all
    float compute runs on device).
  * AGNN without max-subtraction (softmax is shift-invariant; beta*cos is
    bounded), so per graph:
        cos  = hn^T hn + c * ir ir^T      (rank-1 term absorbs the per-graph
                                           broadcast query vector; c=|u_g|^2)
        W    = Ct * exp(beta*cos)         ([src, dst] layout)
        numT = h_nm^T W, den = 1^T W + eps  (PSUM-accumulated matmuls)
        out  = num * recip(den)           (den==0 columns have num==0 -> 0)
  * Zero-in-degree masks (for the broadcast-query MLP term) are topology
    only and computed on host, shipped as a [1, N] row per graph.
  * 32 query graphs (16 nodes each) are packed into one block-diagonal
    512-node graph and use the same code path (c=0, no mask).
  * The query cascade is independent of the data graphs, so it runs first
    for both layers; the per-pair data loop then fuses both layers and
    loads each count matrix once.
  * h state is kept in bf16; node-major copies are produced by the DMA
    crossbar transpose engine (chunk-interleaved: node n -> partition n//4,
    chunk n%4), freeing PE/DVE from transpose work.
All heavy matmuls run in bf16 with fp32 PSUM accumulation.
"""

import threading
from contextlib import ExitStack

import numpy as np
import ml_dtypes

import concourse.bass as bass
import concourse.mybir as mybir
import concourse.tile as tile
from concourse import bacc
from concourse.bass import ds, ts
from concourse.bass_utils import run_bass_kernel_spmd
from concourse.hw_specs import get_activation_tables
from concourse.masks import make_identity

BF16 = mybir.dt.bfloat16
F32 = mybir.dt.float32
AF = mybir.ActivationFunctionType
ALU = mybir.AluOpType

# problem constants
B = 256
NPG = 512
NQPG = 16
IN, H, L, OUT = 64, 128, 2, 1
NCORES = 8
GPC = B // NCORES          # graphs per core (32)
N = NPG                    # dense block size for data graphs
NCH = N // 128             # 4 chunks of 128 src nodes


NQ_FIX = 512               # padded query block (32 graphs x 16 nodes)
G32 = NQ_FIX // NQPG       # 32 query slots


def _act_set_id(arch):
    """Index of an activation-function set containing everything we use."""
    need = {AF.Ln, AF.Exp, AF.Relu, AF.Identity, AF.Copy}
    tabs = get_activation_tables(arch)
    for i, (_, funcs) in enumerate(tabs.items()):
        if need <= funcs:
            return i
    return None


def build_program(gpc=GPC):
    """Build the per-core Bass/Tile program (identical on all 8 cores)."""
    n_nodes = gpc * NPG

    nc = bacc.Bacc("TRN2", target_bir_lowering=False, debug=False,
                   num_devices=NCORES)

    io = {}
    io["xt"] = nc.dram_tensor("xt", [IN, n_nodes], BF16,
                              kind="ExternalInput").ap()
    io["xqt"] = nc.dram_tensor("xqt", [IN, NQ_FIX], BF16,
                               kind="ExternalInput").ap()
    io["ct"] = nc.dram_tensor("ct", [gpc + 1, NCH, 128, N], BF16,
                              kind="ExternalInput").ap()
    io["msk"] = nc.dram_tensor("msk", [gpc, 1, N], BF16,
                               kind="ExternalInput").ap()
    for nm, shp, dt in [
        ("wg", [IN, H], BF16), ("wq", [IN, H], BF16),
        ("wpk", [3 * L + 2, H, H], BF16),   # a1/b1t/w2 per layer, wp1, wp2
        ("bpk", [H, 2 * L + 8], F32),       # all bias/scalar columns
    ]:
        io[nm] = nc.dram_tensor(nm, shp, dt, kind="ExternalInput").ap()
    io["y"] = nc.dram_tensor("y", [1, gpc], F32, kind="ExternalOutput").ap()

    with tile.TileContext(nc) as tc:
        _emit(tc, nc, gpc, io)
    nc.compile()
    return nc


def _emit(tc, nc, gpc, io):
    nq_blk = NQ_FIX

    # keep the full activation table resident: one preloaded set covering
    # Ln/Exp/Relu/Identity/Copy avoids per-iteration table reloads
    set_id = _act_set_id(nc.m.arch)
    if set_id is not None:
        nc.scalar.add_instruction(mybir.InstLoadActFuncSet(
            name=nc.get_next_instruction_name(), ins=[], outs=[],
            act_func_set_id=set_id))

    ctx = ExitStack()
    with ctx:
        pconst = ctx.enter_context(tc.tile_pool(name="pconst", bufs=1))
        pstate = ctx.enter_context(tc.tile_pool(name="pstate", bufs=1))
        pct = ctx.enter_context(tc.tile_pool(name="pct", bufs=4))
        pwork = ctx.enter_context(tc.tile_pool(name="pwork", bufs=6))
        prow = ctx.enter_context(tc.tile_pool(name="prow", bufs=10))
        ps_cos = ctx.enter_context(
            tc.tile_pool(name="ps_cos", bufs=2, space="PSUM"))
        ps_acc = ctx.enter_context(
            tc.tile_pool(name="ps_acc", bufs=3, space="PSUM"))
        ps_row = ctx.enter_context(
            tc.tile_pool(name="ps_row", bufs=3, space="PSUM"))

        def const(name, shape, dtype):
            return pconst.tile(shape, dtype, name=name, tag=name)

        # ---- constants ----
        ident_f = const("ident_f", [128, 128], F32)
        make_identity(nc, ident_f[:])
        ident_bf = const("ident_bf", [128, 128], BF16)
        nc.vector.tensor_copy(ident_bf[:], ident_f[:])
        ones_col_bf = const("ones_col_bf", [128, 1], BF16)
        nc.vector.memset(ones_col_bf[:], 1.0)
        qeps = const("qeps", [1, 1], F32)
        nc.vector.memset(qeps[:], 1e-24)

        # ---- load weights into SBUF ----
        def load(name, ap_dram, shape, dtype):
            t = const(name, shape, dtype)
            nc.sync.dma_start(t[:], ap_dram)
            return t

        wg_s = load("wg_s", io["wg"][:], [IN, H], BF16)
        wq_s = load("wq_s", io["wq"][:], [IN, H], BF16)
        wpk = const("wpk", [128, 3 * L + 2, H], BF16)
        nc.sync.dma_start(wpk[:], io["wpk"].rearrange("k p f -> p k f"))
        bpk = const("bpk", [H, 2 * L + 8], F32)
        nc.sync.dma_start(bpk[:], io["bpk"][:])
        a1_s = [wpk[:, 3 * l + 0, :] for l in range(L)]
        b1t_s = [wpk[:, 3 * l + 1, :] for l in range(L)]
        w2_s = [wpk[:, 3 * l + 2, :] for l in range(L)]
        wp1_s = wpk[:, 3 * L, :]
        wp2_s = wpk[:, 3 * L + 1, 0:1]
        bgc_s = bpk[:, 0:1]
        bqc_s = bpk[:, 1:2]
        b1c_s = [bpk[:, 2 + l:3 + l] for l in range(L)]
        b2c_s = [bpk[:, 2 + L + l:3 + L + l] for l in range(L)]
        bp1c_s = bpk[:, 2 + 2 * L:3 + 2 * L]
        bp2c_s = bpk[0:1, 3 + 2 * L:4 + 2 * L]
        lbg_s = [bpk[0:1, 4 + 2 * L + l:5 + 2 * L + l] for l in range(L)]
        lbq_s = [bpk[0:1, 4 + 3 * L + l:5 + 3 * L + l] for l in range(L)]

        xqt_s = load("xqt_s", io["xqt"][:], [IN, nq_blk], BF16)

        # query-block count matrix: resident for the whole kernel
        ctq_tile = const("ctq_tile", [128, NCH, N], BF16)
        nc.sync.dma_start(ctq_tile[:],
                          io["ct"][gpc].rearrange("c p f -> p c f"))

        # ---- persistent state ----
        def state(name, shape, dtype):
            return pstate.tile(shape, dtype, name=name, tag=name)

        HGT = state("HGT", [128, gpc * N], BF16)          # h_g feature-major
        HGN = state("HGN", [128, gpc * NCH * 128], BF16)  # h_g node-major
        HQT = state("HQT", [128, nq_blk], BF16)
        HQN = state("HQN", [128, NCH * 128], BF16)
        HGS = state("HGS", [128, gpc], F32)
        u_bf = [state(f"u_bf{l}", [128, G32], BF16) for l in range(L)]
        ce_row = [state(f"ce_row{l}", [1, G32], F32) for l in range(L)]
        c_row = [state(f"c_row{l}", [1, G32], F32) for l in range(L)]
        vrow = [state(f"vrow{l}", [1, G32 * 128], BF16) for l in range(L)]

        def hgn_sl(g, sc):
            return HGN[:, ds((g * NCH + sc) * 128, 128)]

        def to_node_major(srcT_ap, dst_ap):
            """[128 feat, 512 node] bf16 -> chunk-interleaved node-major
            (node n -> partition n//4, chunk n%4) via the DMA crossbar."""
            nc.sync.dma_start_transpose(
                dst_ap.rearrange("p (c f) -> p c f", f=128), srcT_ap)

        # ---- initial projections (bias add on DVE: keep the Act queue
        # free for the layer-0 query cascade that gates everything) ----
        xq_ps = ps_acc.tile([128, nq_blk], F32, name="xq_ps", tag="acc")
        nc.tensor.matmul(xq_ps[:], wq_s[:], xqt_s[:], start=True, stop=True)
        nc.vector.tensor_scalar(HQT[:], xq_ps[:], bqc_s, None,
                                op0=ALU.add)
        to_node_major(HQT[:], HQN[:])

        # two passes so the SP DMA queue is never head-of-line blocked by a
        # transpose waiting on compute: all loads+projections stream first,
        # transposes trail as results land. Emitted AFTER query_layer(0)
        # (inside the layer loop) so the layer-0 query cascade — which gates
        # every data-graph head — isn't queued behind 32 bias-adds.
        def init_data_projections():
            for g in range(gpc):
                xg_t = pwork.tile([IN, N], BF16, name="xg_t", tag="xg",
                                  bufs=8)
                nc.sync.dma_start(xg_t[:], io["xt"][:, ts(g, N)])
                xg_ps = ps_acc.tile([128, N], F32, name="xg_ps", tag="acc")
                nc.tensor.matmul(xg_ps[:], wg_s[:], xg_t[:],
                                 start=True, stop=True)
                if g % 2 == 0:
                    nc.vector.tensor_scalar(HGT[:, ts(g, N)], xg_ps[:],
                                            bgc_s, None, op0=ALU.add)
                else:
                    nc.scalar.activation(HGT[:, ts(g, N)], xg_ps[:],
                                         AF.Identity, bias=bgc_s)

        def init_transposes():
            for g in range(gpc):
                to_node_major(HGT[:, ts(g, N)],
                              HGN[:, ds(g * NCH * 128, NCH * 128)])

        def ch(ap_2d, g, sc):
            """Chunk column select: nodes sc*128..sc*128+127 of graph g."""
            return ap_2d[:, ds(g * N + sc * 128, 128)]

        def agnn_graph(den_ps, hn_g, ir_g, cir_g, hgn_of, ct_of):
            """One 512-node dense AGNN: returns num_ps; den accumulated into
            den_ps (a [1,N] row; the host adds 1e-24 to ct row 0 so den>0
            always and zero-in-degree columns give num==0 -> output 0).
            The count matrix is shipped as ln(ct) and added to the cosine
            logits on the PE (identity matmul), so the Exp directly yields
            the masked edge weights: wt = exp(beta*cos + ln ct) = ct*e^bcos.
            beta is folded into hn/ir/cir (hn' = sqrt(beta)*h/|h|).
            hn_g: [128,512] slice (feature-major); ir/cir rows [1,512]
            (cir None for the query block); hgn_of(sc)/ct_of(sc) give
            node-major h and ln-count chunks."""
            num_ps = ps_acc.tile([128, N], F32, name="num_ps", tag="acc")
            for sc in range(NCH):
                cos_ps = ps_cos.tile([128, N], F32, name="cos_ps",
                                     tag="cos")
                nc.tensor.matmul(cos_ps[:], ch(hn_g, 0, sc),
                                 hn_g, start=True, stop=False)
                if cir_g is not None:
                    nc.tensor.matmul(
                        cos_ps[:], cir_g[0:1, ds(sc * 128, 128)],
                        ir_g, start=False, stop=False)
                nc.tensor.matmul(cos_ps[:], ident_bf[:], ct_of(sc),
                                 start=False, stop=True)
                wt = pwork.tile([128, N], BF16, name="wt", tag="ee",
                                bufs=12)
                nc.scalar.activation(wt[:], cos_ps[:], AF.Exp)
                nc.tensor.matmul(num_ps[:], hgn_of(sc), wt[:],
                                 start=(sc == 0), stop=(sc == NCH - 1))
                nc.tensor.matmul(den_ps, ones_col_bf[:], wt[:],
                                 start=(sc == 0), stop=(sc == NCH - 1))
            return num_ps

        # ---- query cascade layer (independent of data graphs) ----
        def query_layer(l):
            sqq = pwork.tile([128, nq_blk], BF16, name="sqq", tag="sq")
            nc.gpsimd.tensor_mul(sqq[:], HQT[:], HQT[:])
            rows_q = ps_row.tile([1, nq_blk], F32, name="rows_q", tag="row")
            nc.tensor.matmul(rows_q[:], ones_col_bf[:], sqq[:],
                             start=True, stop=True)
            lnq = prow.tile([1, nq_blk], F32, name="lnq", tag="frow")
            nc.scalar.activation(lnq[:], rows_q[:], AF.Ln, bias=qeps[:])
            irq = prow.tile([1, nq_blk], BF16, name="irq", tag="brow")
            nc.scalar.activation(irq[:], lnq[:], AF.Exp, scale=-0.5,
                                 bias=lbq_s[l])
            irbq = pwork.tile([128, nq_blk], BF16, name="irbq", tag="irb")
            nc.gpsimd.partition_broadcast(irbq[:], irq[:])
            hnq = pwork.tile([128, nq_blk], BF16, name="hnq", tag="hn")
            nc.vector.tensor_tensor(hnq[:], HQT[:], irbq[:], op=ALU.mult)

            num_ps = agnn_graph(
                rows_q[:], hnq[:], irq[:], None,
                lambda sc: HQN[:, ts(sc, 128)],
                lambda sc: ctq_tile[:, sc, :])
            dmq = prow.tile([1, nq_blk], F32, name="dmq", tag="frow")
            nc.vector.reciprocal(dmq[:], rows_q[:])
            dmqb = pwork.tile([128, nq_blk], F32, name="dmqb", tag="dm")
            nc.gpsimd.partition_broadcast(dmqb[:], dmq[:])
            nc.vector.tensor_tensor(HQT[:], num_ps[:], dmqb[:], op=ALU.mult)
            to_node_major(HQT[:], HQN[:])

            # ---- per-graph query aggregates: u, c = |u|^2+eps, v = B1^T u
            u_f = pwork.tile([128, G32], F32, name="u_f", tag="uf")
            nc.vector.tensor_reduce(
                u_f[:], HQT[:].rearrange("p (g k) -> p g k", k=NQPG),
                axis=mybir.AxisListType.X, op=ALU.add)
            nc.vector.tensor_copy(u_bf[l][:], u_f[:])
            squ = pwork.tile([128, G32], BF16, name="squ", tag="uf")
            nc.gpsimd.tensor_mul(squ[:], u_f[:], u_f[:])
            c_ps = ps_row.tile([1, N], F32, name="c_ps", tag="row")
            nc.tensor.matmul(c_ps[0:1, 0:G32], ones_col_bf[:], squ[:],
                             start=True, stop=True)
            nc.vector.tensor_copy(c_row[l][:], c_ps[0:1, 0:G32])
            nc.vector.tensor_scalar(ce_row[l][:], c_ps[0:1, 0:G32], 1e-24,
                                    None, op0=ALU.add)
            # v = B1^T u for all graphs; flatten [G32,128] rows into a
            # partition-0 row via one SBUF->SBUF DMA
            v_ps = ps_acc.tile([128, N], F32, name="v_ps", tag="acc")
            nc.tensor.matmul(v_ps[:, 0:G32], b1t_s[l], u_bf[l][:],
                             start=True, stop=True)
            v_sb = pwork.tile([128, G32], F32, name="v_sb", tag="uf")
            nc.vector.tensor_copy(v_sb[:], v_ps[:, 0:G32])
            vt_ps = ps_acc.tile([128, N], F32, name="vt_ps", tag="acc")
            nc.tensor.transpose(vt_ps[0:G32, 0:128], v_sb[:], ident_f[:])
            vt32 = pwork.tile([G32, 128], BF16, name="vt32", tag="vt32")
            nc.vector.tensor_copy(vt32[:], vt_ps[0:G32, 0:128])
            nc.sync.dma_start(vrow[l][:], vt32[:])

        # ---- data graphs: layer-major, software-pipelined emission:
        # stage s emits prefetch(s) + head(s) + inner(s-1) + tail(s-2) so
        # every engine queue interleaves adjacent (independent) graphs ----
        def ldct(g):
            ctg = pct.tile([128, NCH, N], BF16, name="ctg", tag="ct")
            nc.sync.dma_start(ctg[:],
                              io["ct"][g].rearrange("c p f -> p c f"))
            mskg = prow.tile([1, N], BF16, name="mskg", tag="msk", bufs=6)
            nc.sync.dma_start(mskg[:], io["msk"][g])
            return ctg, mskg

        def head_a(g, l):
            """|h|^2 row (Pool square + PE ones-matmul)."""
            hT_g = HGT[:, ts(g, N)]
            sqg = pwork.tile([128, N], BF16, name="sqg", tag="sq")
            nc.vector.tensor_tensor(sqg[:], hT_g, hT_g, op=ALU.mult)
            rows = ps_row.tile([1, N], F32, name="rows", tag="row")
            nc.tensor.matmul(rows[:], ones_col_bf[:], sqg[:],
                             start=True, stop=True)
            return rows

        def head_b(g, l, rows):
            """Normalization rows + normalized features (emitted after
            inner(g-1) so the Act queue runs the previous graph's Exp
            before this graph's Ln/Exp)."""
            hT_g = HGT[:, ts(g, N)]
            lnp = prow.tile([1, N], F32, name="lnp", tag="frow")
            # ln(|h|^2 + c_g): per-graph c enters as the bias
            nc.scalar.activation(lnp[:], rows[:], AF.Ln,
                                 bias=ce_row[l][0:1, ds(g, 1)])
            irp = prow.tile([1, N], BF16, name="irp", tag="brow")
            nc.scalar.activation(irp[:], lnp[:], AF.Exp, scale=-0.5,
                                 bias=lbg_s[l])
            cirp = prow.tile([1, N], BF16, name="cirp", tag="brow")
            nc.vector.tensor_scalar(cirp[:], irp[:],
                                    c_row[l][0:1, ds(g, 1)],
                                    None, op0=ALU.mult)
            irbp = pwork.tile([128, N], BF16, name="irbp", tag="irb")
            nc.gpsimd.partition_broadcast(irbp[:], irp[:])
            hng = pwork.tile([128, N], BF16, name="hng", tag="hn")
            nc.vector.tensor_tensor(hng[:], hT_g, irbp[:], op=ALU.mult)
            return irp, cirp, hng

        def inner(g, l, rows, hb, ctg):
            irp, cirp, hng = hb
            return agnn_graph(rows[:], hng[:], irp[:], cirp[:],
                              lambda sc, g=g: hgn_sl(g, sc),
                              lambda sc, ctg=ctg: ctg[:, sc, :])

        def tail(g, l, rows, num_ps, mskg):
            dmr = prow.tile([1, N], F32, name="dmr", tag="frow")
            nc.vector.reciprocal(dmr[:], rows[:])
            dmp = pwork.tile([128, N], F32, name="dmp", tag="dm", bufs=6)
            nc.gpsimd.partition_broadcast(dmp[:], dmr[:])
            s1 = pwork.tile([128, N], BF16, name="s1", tag="s1", bufs=8)
            nc.vector.tensor_tensor(s1[:], num_ps[:], dmp[:], op=ALU.mult)
            z_ps = ps_acc.tile([128, N], F32, name="z_ps", tag="acc")
            nc.tensor.matmul(z_ps[:], a1_s[l], s1[:],
                             start=True, stop=False)
            # broadcast-query MLP term: v_g outer zero-degree mask
            nc.tensor.matmul(z_ps[:], vrow[l][0:1, ts(g, 128)],
                             mskg[:], start=False, stop=True)
            rz = pwork.tile([128, N], BF16, name="rz", tag="s1", bufs=8)
            nc.scalar.activation(rz[:], z_ps[:], AF.Relu, bias=b1c_s[l])
            h2_ps = ps_acc.tile([128, N], F32, name="h2_ps", tag="acc")
            nc.tensor.matmul(h2_ps[:], w2_s[l], rz[:],
                             start=True, stop=True)
            if l == L - 1:
                nc.vector.tensor_scalar(
                    HGT[:, ts(g, N)], h2_ps[:], b2c_s[l], 0.0,
                    op0=ALU.add, op1=ALU.add,
                    accum_out=HGS[:, ds(g, 1)])
            else:
                nc.vector.tensor_scalar(
                    HGT[:, ts(g, N)], h2_ps[:], b2c_s[l], None,
                    op0=ALU.add)
                to_node_major(HGT[:, ts(g, N)],
                              HGN[:, ds(g * NCH * 128, NCH * 128)])

        # one continuous software-pipelined stream over all (layer, graph)
        # stages: the layer boundary neither drains nor refills the pipeline
        query_layer(0)
        init_data_projections()
        NS = L * gpc
        cts = {}
        rws = {}
        hbs = {}
        nums = {}
        # prefetch the first stages' count matrices BEFORE the init
        # transposes hit the SP queue (they wait on compute and would
        # head-of-line block these loads)
        for s in range(4):
            cts[s] = ldct(s)
        init_transposes()
        for s in range(NS + 2):
            for l in range(1, L):
                if s == (2 * l - 1) * gpc // 2:
                    # emit the next query layer mid-stream: it only depends
                    # on the previous query layer, so it fills pipeline
                    # slack and its aggregates are ready well before the
                    # next data layer starts
                    query_layer(l)
            if s < NS:
                if s not in cts:
                    cts[s] = ldct(s % gpc)
                rws[s] = head_a(s % gpc, s // gpc)
            if 1 <= s <= NS:
                p = s - 1
                nums[p] = inner(p % gpc, p // gpc, rws[p], hbs[p],
                                cts[p][0])
            if s < NS:
                hbs[s] = head_b(s % gpc, s // gpc, rws[s])
            if s >= 2:
                p = s - 2
                tail(p % gpc, p // gpc, rws[p], nums[p], cts[p][1])
                del nums[p], hbs[p], rws[p], cts[p]

        # ---- final predictor ----
        hgs_bf = pwork.tile([128, gpc], BF16, name="hgs_bf", tag="uf")
        nc.vector.tensor_copy(hgs_bf[:], HGS[:])
        z1_ps = ps_acc.tile([128, N], F32, name="z1_ps", tag="acc")
        nc.tensor.matmul(z1_ps[:, 0:gpc], wp1_s, hgs_bf[:],
                         start=True, stop=True)
        r1 = pwork.tile([128, gpc], BF16, name="r1", tag="uf")
        nc.scalar.activation(r1[:], z1_ps[:, 0:gpc], AF.Relu, bias=bp1c_s)
        y_ps = ps_row.tile([1, N], F32, name="y_ps", tag="row")
        nc.tensor.matmul(y_ps[0:1, 0:gpc], wp2_s, r1[:],
                         start=True, stop=True)
        y_sb = prow.tile([1, gpc], F32, name="y_sb", tag="frow")
        nc.scalar.activation(y_sb[:], y_ps[0:1, 0:gpc], AF.Identity,
                             bias=bp2c_s)
        nc.sync.dma_start(io["y"][:], y_sb[:])


def _build_ct_np(src, dst, npb, nblocks):
    blk = src // npb
    s = src - blk * npb
    d = dst - blk * npb
    flat = blk * (npb * npb) + s * npb + d
    cnt = np.bincount(flat, minlength=nblocks * npb * npb)
    return cnt.reshape(nblocks, npb, npb)


def _chunk_rows(ct):
    """[nb, 512, 512] -> [nb, 4, 128, 512]: chunk c holds src rows
    c*128..c*128+127 (matches the DMA-crossbar transpose layout)."""
    nb = ct.shape[0]
    return np.ascontiguousarray(ct.reshape(nb, NCH, 128, N))


_PROG_CACHE = {}
_PROG_LOCK = threading.Lock()


def _get_program(gpc=GPC):
    with _PROG_LOCK:
        if gpc not in _PROG_CACHE:
            _PROG_CACHE[gpc] = build_program(gpc)
        return _PROG_CACHE[gpc]


def _make_in_maps(inputs, gpc=GPC, ncores=NCORES):
    bf = ml_dtypes.bfloat16
    X = np.asarray(inputs["X"], np.float32)
    X_q = np.asarray(inputs["X_q"], np.float32)
    g_src = np.asarray(inputs["g_src"], np.int64)
    g_dst = np.asarray(inputs["g_dst"], np.int64)
    q_src = np.asarray(inputs["q_src"], np.int64)
    q_dst = np.asarray(inputs["q_dst"], np.int64)

    W1r = np.asarray(inputs["W1r"], np.float32)
    W2r = np.asarray(inputs["W2r"], np.float32)
    wpk = np.zeros((3 * L + 2, H, H), np.float32)
    for l in range(L):
        wpk[3 * l + 0] = W1r[l, :H, :]
        wpk[3 * l + 1] = W1r[l, H:, :]
        wpk[3 * l + 2] = W2r[l]
    wpk[3 * L] = np.asarray(inputs["Wp1"], np.float32)
    wpk[3 * L + 1, :, 0:1] = np.asarray(inputs["Wp2"], np.float32)

    bpk = np.zeros((H, 2 * L + 8), np.float32)
    bpk[:, 0] = np.asarray(inputs["bg"], np.float32)
    bpk[:, 1] = np.asarray(inputs["bq"], np.float32)
    for l in range(L):
        bpk[:, 2 + l] = np.asarray(inputs["b1r"], np.float32)[l]
        bpk[:, 2 + L + l] = np.asarray(inputs["b2r"], np.float32)[l]
    bpk[:, 2 + 2 * L] = np.asarray(inputs["bp1"], np.float32)
    bpk[0, 3 + 2 * L] = float(np.asarray(inputs["bp2"], np.float32)[0])
    lbg = 0.5 * np.log(np.asarray(inputs["betas_g"], np.float32))
    lbq = 0.5 * np.log(np.asarray(inputs["betas_q"], np.float32))
    for l in range(L):
        bpk[0, 4 + 2 * L + l] = lbg[l]
        bpk[0, 4 + 3 * L + l] = lbq[l]

    shared = {
        "wg": np.asarray(inputs["Wg"], np.float32).astype(bf),
        "wq": np.asarray(inputs["Wq"], np.float32).astype(bf),
        "wpk": wpk.astype(bf),
        "bpk": bpk,
    }

    n = gpc * NPG
    nq = gpc * NQPG
    ne = n * 8
    nqe = nq * 8
    in_maps = []
    for c in range(ncores):
        xc = X[c * n:(c + 1) * n]
        xqc = X_q[c * nq:(c + 1) * nq]
        gs = g_src[c * ne:(c + 1) * ne] - c * n
        gd = g_dst[c * ne:(c + 1) * ne] - c * n
        qs = q_src[c * nqe:(c + 1) * nqe] - c * nq
        qd = q_dst[c * nqe:(c + 1) * nqe] - c * nq

        ct_g = _build_ct_np(gs, gd, NPG, gpc)       # [gpc, 512, 512]
        ct_q = _build_ct_np(qs, qd, NQPG, gpc)      # [gpc, 16, 16]
        ctq_blk = np.zeros((512, 512), np.int64)
        for g in range(gpc):
            ctq_blk[g * NQPG:(g + 1) * NQPG,
                    g * NQPG:(g + 1) * NQPG] = ct_q[g]

        msk = (ct_g.sum(axis=1) > 0).astype(bf).reshape(gpc, 1, NPG)

        ct_all = np.concatenate([ct_g, ctq_blk[None]], 0).astype(np.float64)
        # tiny weight on src row 0 keeps den > 0 everywhere (so recip never
        # sees 0) while perturbing num by ~1e-24
        ct_all[:, 0, :] += 1e-24
        # ship ln(ct): exp(cos + ln ct) = ct * e^cos; ct==0 -> -300
        # (exp underflows to 0)
        with np.errstate(divide="ignore"):
            lct = np.log(ct_all)
        lct[np.isneginf(lct)] = -300.0
        ct_all = _chunk_rows(lct).astype(bf)

        m = dict(shared)
        m["xt"] = np.ascontiguousarray(xc.T).astype(bf)
        xqt = np.zeros((IN, 512), np.float32)
        xqt[:, :nq] = xqc.T
        m["xqt"] = xqt.astype(bf)
        m["ct"] = ct_all
        m["msk"] = msk
        in_maps.append(m)
    return in_maps


def run(inputs, trace=False, gpc=GPC):
    nc = _get_program(gpc)
    in_maps = _make_in_maps(inputs, gpc=gpc)
    res = run_bass_kernel_spmd(nc, in_maps, list(range(NCORES)), trace=trace)
    ys = [res.results[c]["y"].reshape(-1) for c in range(NCORES)]
    out = np.concatenate(ys).astype(np.float32).reshape(B, OUT)
    return out, res


def kernel(**inputs) -> np.ndarray:
    out, _ = run(inputs, trace=False)
    return out
